# revision 3
# baseline (speedup 1.0000x reference)
"""Multi-head attention (B=2, S=2048, D=1024, H=16) on 8 Trainium2 cores.

Sharding: core = b*4 + g  ->  batch b (data parallel), head-group g of 4
heads (tensor parallel).  Each core computes a partial out^T = Wo_g^T @ Z_g
for its batch; the host sums the 4 partials per batch (the "all-reduce"),
transposes back and adds the (folded) output bias.

All activations flow feature-major on device (x^T, Q^T, K^T, scores^T) so
no on-device transposes are needed.  Matmuls run in bf16 with fp32 PSUM
accumulation.  Softmax skips the row-max pass (scores are bounded), gets
its denominator from a ones-column appended to V, and defers normalization
to after the attention*V matmul.
"""

import numpy as np
import ml_dtypes

B, S, D, H = 2, 2048, 1024, 16
DK = D // H                  # 64
SCALE = 1.0 / np.sqrt(D)
NCORES = 8
GROUPS = 4                   # head-groups (tensor parallel)
HG = H // GROUPS             # 4 heads per group
DG = D // GROUPS             # 256 head dims per group
P = 128
KO = D // P                  # 8 contraction chunks for the projections
MO = DG // P                 # 2 row-chunks of Q^T/K^T (= head pairs)
NQ = 512                     # q tile width
QT = S // NQ                 # 4
ST = S // P                  # 16 key blocks / s chunks
BF16 = ml_dtypes.bfloat16

_cache = {}


def _classify_mask(mask):
    """Block structure of mask^T ([k, q] layout, P x NQ blocks).

    Returns (cls, qoff, mixed_idx, mixed_tiles, use_affine):
      cls[kt][qt]  : 0 all-masked, 1 all-kept, 2 mixed
      qoff[kt][qt] : leading all-masked columns (trim), 0 unless tril
      mixed_idx    : {(kt, qt): index into mixed_tiles}
      mixed_tiles  : np [n, P, NQ] bf16 0/1 tiles (empty when use_affine)
    """
    tril = np.tril(np.ones((S, S), dtype=mask.dtype))
    use_affine = bool(np.array_equal(mask, tril))
    cls = [[1] * QT for _ in range(ST)]
    qoff = [[0] * QT for _ in range(ST)]
    mixed_idx = {}
    tiles = []
    if use_affine:
        for kt in range(ST):
            k0 = kt * P
            for qt in range(QT):
                q0 = qt * NQ
                if k0 - q0 >= NQ:
                    cls[kt][qt] = 0
                elif k0 + P - 1 > q0:
                    cls[kt][qt] = 2
                    qoff[kt][qt] = min(max(k0 - q0, 0), NQ - P)
                # else: fully kept
    else:
        keepT = (mask != 0).T        # [k, q]
        for kt in range(ST):
            for qt in range(QT):
                blk = keepT[kt * P:(kt + 1) * P, qt * NQ:(qt + 1) * NQ]
                if not blk.any():
                    cls[kt][qt] = 0
                elif blk.all():
                    cls[kt][qt] = 1
                else:
                    cls[kt][qt] = 2
                    mixed_idx[(kt, qt)] = len(tiles)
                    tiles.append(blk.astype(BF16))
    mixed_tiles = (np.stack(tiles) if tiles else
                   np.zeros((0, P, NQ), dtype=BF16))
    return cls, qoff, mixed_idx, mixed_tiles, use_affine


def _build_program(cls, qoff, mixed_idx, n_mixed, use_affine):
    from contextlib import ExitStack
    import concourse.bass as bass
    import concourse.tile as tile
    import concourse.mybir as mybir
    from concourse import bacc
    from concourse.bass import ds, ts

    f32 = mybir.dt.float32
    bf16 = mybir.dt.bfloat16
    Exp = mybir.ActivationFunctionType.Exp

    nc = bacc.Bacc(None, target_bir_lowering=False, name="mha_tp")

    xT = nc.dram_tensor("xT", [D, S], bf16, kind="ExternalInput")
    wq = nc.dram_tensor("wq", [D, DG], bf16, kind="ExternalInput")
    wk = nc.dram_tensor("wk", [D, DG], bf16, kind="ExternalInput")
    wv = nc.dram_tensor("wv", [D, DG], bf16, kind="ExternalInput")
    wo = nc.dram_tensor("wo", [DG, D], bf16, kind="ExternalInput")
    bqk = nc.dram_tensor("bqk", [2, DG], f32, kind="ExternalInput")
    mm = (nc.dram_tensor("mmask", [n_mixed, P, NQ], bf16, kind="ExternalInput")
          if n_mixed else None)
    outT = nc.dram_tensor("outT", [D, S], f32, kind="ExternalOutput")

    xTv = xT.ap().rearrange("(ko p) s -> p ko s", p=P)
    wqv = wq.ap().rearrange("(ko p) m -> p ko m", p=P)
    wkv = wk.ap().rearrange("(ko p) m -> p ko m", p=P)
    wvv = wv.ap().rearrange("(ko p) m -> p ko m", p=P)
    wov = wo.ap().rearrange("(zo p) n -> p zo n", p=P)
    bqkv = bqk.ap().rearrange("t (mo p) -> p t mo", p=P)
    outv = outT.ap().rearrange("(mo p) s -> p mo s", p=P)

    with tile.TileContext(nc) as tc, ExitStack() as ctx:
        const = ctx.enter_context(tc.tile_pool(name="const", bufs=1))

        x_sb = const.tile([P, KO, S], bf16)
        for ko in range(KO):
            nc.sync.dma_start(x_sb[:, ko, :], xTv[:, ko, :])
        wq_sb = const.tile([P, KO, DG], bf16)
        nc.sync.dma_start(wq_sb[:], wqv)
        wk_sb = const.tile([P, KO, DG], bf16)
        nc.sync.dma_start(wk_sb[:], wkv)
        wv_sb = const.tile([P, KO, DG], bf16)
        nc.sync.dma_start(wv_sb[:], wvv)
        wo_sb = const.tile([P, MO, D], bf16)
        nc.sync.dma_start(wo_sb[:], wov)
        bias_sb = const.tile([P, 2, 2], f32)
        nc.sync.dma_start(bias_sb[:], bqkv)
        mask_sb = None
        if n_mixed:
            mask_sb = const.tile([P, n_mixed, NQ], bf16)
            for i in range(n_mixed):
                nc.sync.dma_start(mask_sb[:, i, :], mm.ap()[i])

        qT_sb = const.tile([P, MO, S], bf16)
        kT_sb = const.tile([P, MO, S], bf16)
        v_sb = const.tile([P, ST, HG, DK + 1], bf16)
        zT_sb = const.tile([P, MO, S], bf16)
        nc.gpsimd.memset(v_sb[:, :, :, DK:DK + 1], 1.0)

        # ---- QKV projections -------------------------------------------
        with tc.tile_pool(name="pqkv", bufs=4, space="PSUM") as pqkv:
            for t, (w_sb, dst) in enumerate(((wq_sb, qT_sb), (wk_sb, kT_sb))):
                for mo in range(MO):
                    for qt in range(QT):
                        ps = pqkv.tile([P, NQ], f32, tag="ps")
                        for ko in range(KO):
                            nc.tensor.matmul(
                                ps, w_sb[:, ko, ts(mo, P)],
                                x_sb[:, ko, ts(qt, NQ)],
                                start=(ko == 0), stop=(ko == KO - 1))
                        nc.vector.tensor_scalar_add(
                            dst[:, mo, ts(qt, NQ)], ps,
                            bias_sb[:, t, mo:mo + 1])
            for so in range(ST):
                ps = pqkv.tile([P, DG], f32, tag="psv")
                for ko in range(KO):
                    nc.tensor.matmul(
                        ps, x_sb[:, ko, ts(so, P)], wv_sb[:, ko, :],
                        start=(ko == 0), stop=(ko == KO - 1))
                nc.vector.tensor_copy(
                    v_sb[:, so, :, 0:DK],
                    ps.rearrange("p (h d) -> p h d", h=HG))

        # ---- attention + output projection -----------------------------
        with (
            tc.tile_pool(name="ps_at", bufs=2, space="PSUM") as ps_at,
            tc.tile_pool(name="pz", bufs=4, space="PSUM") as pz,
            tc.tile_pool(name="po", bufs=2, space="PSUM") as po,
            tc.tile_pool(name="work", bufs=6) as work,
            tc.tile_pool(name="rwork", bufs=4) as rwork,
            tc.tile_pool(name="dscr", bufs=4, space="DRAM") as dscr,
        ):
            for qt in range(QT):
                q0 = qt * NQ
                for mo in range(MO):
                    kts = [kt for kt in range(ST) if cls[kt][qt] != 0]
                    if not kts:
                        nc.vector.memset(zT_sb[:, mo, ts(qt, NQ)], 0.0)
                        continue
                    z01 = [pz.tile([P, NQ], f32, tag="z", name=f"z{hh}")
                           for hh in range(2)]
                    for i, kt in enumerate(kts):
                        k0 = kt * P
                        off = qoff[kt][qt]
                        w = NQ - off
                        first, last = i == 0, i == len(kts) - 1
                        for h in (0, 1):
                            hp = slice(h * DK, (h + 1) * DK)
                            s_ps = ps_at.tile([P, NQ], f32, tag="s")
                            nc.tensor.matmul(
                                s_ps[:, off:],
                                kT_sb[hp, mo, ts(kt, P)],
                                qT_sb[hp, mo, ds(q0 + off, w)],
                                start=True, stop=True)
                            pT = work.tile([P, NQ], bf16, tag="pT")
                            nc.scalar.activation(pT[:, off:], s_ps[:, off:], Exp)
                            if cls[kt][qt] == 2:
                                if use_affine:
                                    nc.gpsimd.affine_select(
                                        out=pT[:, off:], in_=pT[:, off:],
                                        compare_op=mybir.AluOpType.is_ge,
                                        fill=0.0,
                                        base=q0 + off - k0,
                                        channel_multiplier=-1,
                                        pattern=[[1, w]])
                                else:
                                    nc.vector.tensor_mul(
                                        pT[:, off:], pT[:, off:],
                                        mask_sb[:, mixed_idx[(kt, qt)], off:])
                            nc.tensor.matmul(
                                z01[h][0:DK + 1, off:],
                                v_sb[:, kt, 2 * mo + h, :],
                                pT[:, off:],
                                start=first, stop=last)
                    for h in (0, 1):
                        z = z01[h]
                        r_sb = rwork.tile([DK + 1, NQ], f32, tag="r")
                        nc.vector.reciprocal(r_sb[DK:DK + 1, :], z[DK:DK + 1, :])
                        r_dr = dscr.tile([1, NQ], f32, tag="rd")
                        nc.sync.dma_start(r_dr[:], r_sb[DK:DK + 1, :])
                        rb = rwork.tile([DK, NQ], f32, tag="rb")
                        nc.sync.dma_start(rb[:], r_dr.to_broadcast((DK, NQ)))
                        if h == 0:
                            nc.vector.tensor_mul(
                                zT_sb[0:DK, mo, ts(qt, NQ)], z[0:DK, :], rb[:])
                        else:
                            zn_tmp = rwork.tile([DK, NQ], bf16, tag="zt")
                            nc.vector.tensor_mul(zn_tmp[:], z[0:DK, :], rb[:])
                            nc.sync.dma_start(
                                zT_sb[DK:P, mo, ts(qt, NQ)], zn_tmp[:])
                # output projection for this q tile
                for mo8 in range(D // P):
                    o_ps = po.tile([P, NQ], f32, tag="o")
                    for zo in range(MO):
                        nc.tensor.matmul(
                            o_ps, wo_sb[:, zo, ts(mo8, P)],
                            zT_sb[:, zo, ts(qt, NQ)],
                            start=(zo == 0), stop=(zo == MO - 1))
                    o_sb = work.tile([P, NQ], f32, tag="osb")
                    nc.vector.tensor_copy(o_sb[:], o_ps)
                    nc.sync.dma_start(outv[:, mo8, ts(qt, NQ)], o_sb[:])
    return nc


def _get_program(mask):
    cls, qoff, mixed_idx, mixed_tiles, use_affine = _classify_mask(mask)
    key = (use_affine,
           tuple(tuple(r) for r in cls),
           tuple(tuple(r) for r in qoff))
    if key not in _cache:
        nc = _build_program(cls, qoff, mixed_idx, len(mixed_tiles), use_affine)
        nc.compile()
        _cache[key] = nc
    return _cache[key], mixed_tiles


def _prep_in_maps(x, mask, Wq, bq, Wk, bk, Wv, bv, Wo, bo, mixed_tiles):
    xT = [np.ascontiguousarray(x[b].T).astype(BF16) for b in range(B)]
    in_maps = []
    for core in range(NCORES):
        b, g = divmod(core, GROUPS)
        c0, c1 = g * DG, (g + 1) * DG
        im = {
            "xT": xT[b],
            "wq": np.ascontiguousarray(Wq[:, c0:c1] * SCALE).astype(BF16),
            "wk": np.ascontiguousarray(Wk[:, c0:c1]).astype(BF16),
            "wv": np.ascontiguousarray(Wv[:, c0:c1]).astype(BF16),
            "wo": np.ascontiguousarray(Wo[c0:c1, :]).astype(BF16),
            "bqk": np.ascontiguousarray(
                np.stack([bq[c0:c1] * SCALE, bk[c0:c1]])).astype(np.float32),
        }
        if len(mixed_tiles):
            im["mmask"] = mixed_tiles
        in_maps.append(im)
    return in_maps


def _unshard(results, Wo, bv, bo):
    bo_eff = (bo.astype(np.float32)
              + bv.astype(np.float32) @ Wo.astype(np.float32))
    out = np.empty((B, S, D), np.float32)
    for b in range(B):
        acc = results[b * GROUPS]["outT"].astype(np.float32).copy()
        for g in range(1, GROUPS):
            acc += results[b * GROUPS + g]["outT"]
        out[b] = acc.T + bo_eff
    return out


def kernel(trace=False, **inputs):
    from concourse import bass_utils

    args = {k: np.asarray(v) for k, v in inputs.items()}
    x, mask = args["x"], args["mask"]
    Wq, bq = args["Wq"], args["bq"]
    Wk, bk = args["Wk"], args["bk"]
    Wv, bv = args["Wv"], args["bv"]
    Wo, bo = args["Wo"], args["bo"]

    nc, mixed_tiles = _get_program(mask)
    in_maps = _prep_in_maps(x, mask, Wq, bq, Wk, bk, Wv, bv, Wo, bo,
                            mixed_tiles)
    res = bass_utils.run_bass_kernel_spmd(
        nc, in_maps, core_ids=list(range(NCORES)), trace=trace)
    out = _unshard(res.results, Wo, bv, bo)
    kernel.last_results = res
    return out


# revision 8
# speedup vs baseline: 1.0642x; 1.0642x over previous
"""Multi-head attention (B=2, S=2048, D=1024, H=16) on 8 Trainium2 cores.

Sharding: core = b*4 + g  ->  batch b (data parallel), head-group g of 4
heads (tensor parallel).  Each core computes a partial out^T = Wo_g^T @ Z_g
for its batch; the host sums the 4 partials per batch (the "all-reduce"),
transposes back and adds the (folded) output bias.

All activations flow feature-major on device (x^T, Q^T, K^T, scores^T) so
no on-device transposes are needed.  Matmuls run in bf16 with fp32 PSUM
accumulation.  Softmax skips the row-max pass (scores are bounded), gets
its denominator from a ones-column appended to V, and defers normalization
to after the attention*V matmul.
"""

import numpy as np
import ml_dtypes

B, S, D, H = 2, 2048, 1024, 16
DK = D // H                  # 64
SCALE = 1.0 / np.sqrt(D)
NCORES = 8
GROUPS = 4                   # head-groups (tensor parallel)
HG = H // GROUPS             # 4 heads per group
DG = D // GROUPS             # 256 head dims per group
P = 128
KO = D // P                  # 8 contraction chunks for the projections
MO = DG // P                 # 2 row-chunks of Q^T/K^T (= head pairs)
NQ = 512                     # q tile width
QT = S // NQ                 # 4
ST = S // P                  # 16 key blocks / s chunks
BF16 = ml_dtypes.bfloat16

_cache = {}


def _classify_mask(mask):
    """Block structure of mask^T ([k, q] layout, P x NQ blocks).

    Returns (cls, qoff, mixed_idx, mixed_tiles, use_affine):
      cls[kt][qt]  : 0 all-masked, 1 all-kept, 2 mixed
      qoff[kt][qt] : leading all-masked columns (trim), 0 unless tril
      mixed_idx    : {(kt, qt): index into mixed_tiles}
      mixed_tiles  : np [n, P, NQ] bf16 0/1 tiles (empty when use_affine)
    """
    tril = np.tril(np.ones((S, S), dtype=mask.dtype))
    use_affine = bool(np.array_equal(mask, tril))
    cls = [[1] * QT for _ in range(ST)]
    qoff = [[0] * QT for _ in range(ST)]
    mixed_idx = {}
    tiles = []
    if use_affine:
        for kt in range(ST):
            k0 = kt * P
            for qt in range(QT):
                q0 = qt * NQ
                if k0 - q0 >= NQ:
                    cls[kt][qt] = 0
                elif k0 + P - 1 > q0:
                    cls[kt][qt] = 2
                    qoff[kt][qt] = min(max(k0 - q0, 0), NQ - P)
                # else: fully kept
    else:
        keepT = (mask != 0).T        # [k, q]
        for kt in range(ST):
            for qt in range(QT):
                blk = keepT[kt * P:(kt + 1) * P, qt * NQ:(qt + 1) * NQ]
                if not blk.any():
                    cls[kt][qt] = 0
                elif blk.all():
                    cls[kt][qt] = 1
                else:
                    cls[kt][qt] = 2
                    mixed_idx[(kt, qt)] = len(tiles)
                    tiles.append(blk.astype(BF16))
    mixed_tiles = (np.stack(tiles) if tiles else
                   np.zeros((0, P, NQ), dtype=BF16))
    return cls, qoff, mixed_idx, mixed_tiles, use_affine


def _build_program(cls, qoff, mixed_idx, n_mixed, use_affine):
    from contextlib import ExitStack
    import concourse.bass as bass
    import concourse.tile as tile
    import concourse.mybir as mybir
    from concourse import bacc
    from concourse.bass import ds, ts

    f32 = mybir.dt.float32
    bf16 = mybir.dt.bfloat16
    Exp = mybir.ActivationFunctionType.Exp

    nc = bacc.Bacc(None, target_bir_lowering=False, name="mha_tp")

    xT = nc.dram_tensor("xT", [D, S], bf16, kind="ExternalInput")
    wq = nc.dram_tensor("wq", [D, DG], bf16, kind="ExternalInput")
    wk = nc.dram_tensor("wk", [D, DG], bf16, kind="ExternalInput")
    wv = nc.dram_tensor("wv", [D, DG], bf16, kind="ExternalInput")
    wo = nc.dram_tensor("wo", [DG, D], bf16, kind="ExternalInput")
    bqk = nc.dram_tensor("bqk", [2, DG], f32, kind="ExternalInput")
    mm = (nc.dram_tensor("mmask", [n_mixed, P, NQ], bf16, kind="ExternalInput")
          if n_mixed else None)
    outT = nc.dram_tensor("outT", [D, S], f32, kind="ExternalOutput")

    xTv = xT.ap().rearrange("(ko p) s -> p ko s", p=P)
    wqv = wq.ap().rearrange("(ko p) m -> p ko m", p=P)
    wkv = wk.ap().rearrange("(ko p) m -> p ko m", p=P)
    wvv = wv.ap().rearrange("(ko p) m -> p ko m", p=P)
    wov = wo.ap().rearrange("(zo p) n -> p zo n", p=P)
    bqkv = bqk.ap().rearrange("t (mo p) -> p t mo", p=P)
    outv = outT.ap().rearrange("(mo p) s -> p mo s", p=P)

    with tile.TileContext(nc) as tc, ExitStack() as ctx:
        const = ctx.enter_context(tc.tile_pool(name="const", bufs=1))

        x_sb = const.tile([P, KO, S], bf16)
        for ko in range(KO):
            nc.sync.dma_start(x_sb[:, ko, :], xTv[:, ko, :])
        wq_sb = const.tile([P, KO, DG], bf16)
        nc.sync.dma_start(wq_sb[:], wqv)
        wk_sb = const.tile([P, KO, DG], bf16)
        nc.sync.dma_start(wk_sb[:], wkv)
        wv_sb = const.tile([P, KO, DG], bf16)
        nc.sync.dma_start(wv_sb[:], wvv)
        wo_sb = const.tile([P, MO, D], bf16)
        nc.sync.dma_start(wo_sb[:], wov)
        bias_sb = const.tile([P, 2, 2], f32)
        nc.sync.dma_start(bias_sb[:], bqkv)
        mask_sb = None
        if n_mixed:
            mask_sb = const.tile([P, n_mixed, NQ], bf16)
            for i in range(n_mixed):
                nc.sync.dma_start(mask_sb[:, i, :], mm.ap()[i])

        qT_sb = const.tile([P, MO, S], bf16)
        kT_sb = const.tile([P, MO, S], bf16)
        v_sb = const.tile([P, ST, HG, DK + 1], bf16)
        zT_sb = const.tile([P, MO, S], bf16)
        nc.gpsimd.memset(v_sb[:, :, :, DK:DK + 1], 1.0)

        # ---- QKV projections -------------------------------------------
        with tc.tile_pool(name="pqkv", bufs=4, space="PSUM") as pqkv:
            for t, (w_sb, dst) in enumerate(((wq_sb, qT_sb), (wk_sb, kT_sb))):
                for mo in range(MO):
                    for qt in range(QT):
                        ps = pqkv.tile([P, NQ], f32, tag="ps")
                        for ko in range(KO):
                            nc.tensor.matmul(
                                ps, w_sb[:, ko, ts(mo, P)],
                                x_sb[:, ko, ts(qt, NQ)],
                                start=(ko == 0), stop=(ko == KO - 1))
                        nc.vector.tensor_scalar_add(
                            dst[:, mo, ts(qt, NQ)], ps,
                            bias_sb[:, t, mo:mo + 1])
            for so in range(ST):
                ps = pqkv.tile([P, DG], f32, tag="psv")
                for ko in range(KO):
                    nc.tensor.matmul(
                        ps, x_sb[:, ko, ts(so, P)], wv_sb[:, ko, :],
                        start=(ko == 0), stop=(ko == KO - 1))
                nc.vector.tensor_copy(
                    v_sb[:, so, :, 0:DK],
                    ps.rearrange("p (h d) -> p h d", h=HG))

        # ---- attention + output projection -----------------------------
        with (
            tc.tile_pool(name="ps_at", bufs=2, space="PSUM") as ps_at,
            tc.tile_pool(name="pz", bufs=4, space="PSUM") as pz,
            tc.tile_pool(name="po", bufs=2, space="PSUM") as po,
            tc.tile_pool(name="work", bufs=6) as work,
            tc.tile_pool(name="rwork", bufs=3) as rwork,
            tc.tile_pool(name="dscr", bufs=3, space="DRAM") as dscr,
        ):
            for qt in range(QT):
                q0 = qt * NQ
                for mo in range(MO):
                    kts = [kt for kt in range(ST) if cls[kt][qt] != 0]
                    if not kts:
                        nc.vector.memset(zT_sb[:, mo, ts(qt, NQ)], 0.0)
                        continue
                    z01 = [pz.tile([P, NQ], f32, tag="z", name=f"z{hh}")
                           for hh in range(2)]
                    for i, kt in enumerate(kts):
                        k0 = kt * P
                        off = qoff[kt][qt]
                        w = NQ - off
                        first, last = i == 0, i == len(kts) - 1
                        for h in (0, 1):
                            hp = slice(h * DK, (h + 1) * DK)
                            s_ps = ps_at.tile([P, NQ], f32, tag="s")
                            nc.tensor.matmul(
                                s_ps[:, off:],
                                kT_sb[hp, mo, ts(kt, P)],
                                qT_sb[hp, mo, ds(q0 + off, w)],
                                start=True, stop=True)
                            pT = work.tile([P, NQ], bf16, tag="pT")
                            nc.scalar.activation(pT[:, off:], s_ps[:, off:], Exp)
                            if cls[kt][qt] == 2:
                                if use_affine:
                                    nc.gpsimd.affine_select(
                                        out=pT[:, off:], in_=pT[:, off:],
                                        compare_op=mybir.AluOpType.is_ge,
                                        fill=0.0,
                                        base=q0 + off - k0,
                                        channel_multiplier=-1,
                                        pattern=[[1, w]])
                                else:
                                    nc.vector.tensor_mul(
                                        pT[:, off:], pT[:, off:],
                                        mask_sb[:, mixed_idx[(kt, qt)], off:])
                            nc.tensor.matmul(
                                z01[h][0:DK + 1, off:],
                                v_sb[:, kt, 2 * mo + h, :],
                                pT[:, off:],
                                start=first, stop=last)
                    # softmax denominators: collect both heads' rows, spread
                    # them across all 128 lanes via DRAM for a cheap
                    # reciprocal, then broadcast back over partitions.
                    den_sb = rwork.tile([DK + 1, 2, NQ], f32, tag="den")
                    for h in (0, 1):
                        nc.vector.tensor_copy(
                            den_sb[DK:DK + 1, h, :], z01[h][DK:DK + 1, :])
                    NJ = 2 * NQ // P                      # 8 elems per lane
                    d_dr = dscr.tile([2, NQ], f32, tag="dd")
                    nc.scalar.dma_start(d_dr[:], den_sb[DK:DK + 1, :, :])
                    d_sp = rwork.tile([P, NJ], f32, tag="dsp")
                    nc.scalar.dma_start(
                        d_sp[:], d_dr.rearrange("h (a b) -> (h a) b", b=NJ))
                    r_sp = rwork.tile([P, NJ], f32, tag="rsp")
                    nc.vector.reciprocal(r_sp[:], d_sp[:])
                    r_dr = dscr.tile([2, NQ], f32, tag="rd")
                    nc.sync.dma_start(
                        r_dr.rearrange("h (a b) -> (h a) b", b=NJ), r_sp[:])
                    rb = rwork.tile([DK, 2, NQ], f32, tag="rb")
                    nc.sync.dma_start(rb[:], r_dr[None].to_broadcast((DK, 2, NQ)))
                    nc.vector.tensor_mul(
                        zT_sb[0:DK, mo, ts(qt, NQ)], z01[0][0:DK, :],
                        rb[:, 0, :])
                    zn_tmp = rwork.tile([DK, NQ], bf16, tag="zt")
                    nc.vector.tensor_mul(zn_tmp[:], z01[1][0:DK, :],
                                         rb[:, 1, :])
                    nc.sync.dma_start(zT_sb[DK:P, mo, ts(qt, NQ)], zn_tmp[:])
                # output projection for this q tile
                for mo8 in range(D // P):
                    o_ps = po.tile([P, NQ], f32, tag="o")
                    for zo in range(MO):
                        nc.tensor.matmul(
                            o_ps, wo_sb[:, zo, ts(mo8, P)],
                            zT_sb[:, zo, ts(qt, NQ)],
                            start=(zo == 0), stop=(zo == MO - 1))
                    o_sb = work.tile([P, NQ], f32, tag="osb")
                    nc.vector.tensor_copy(o_sb[:], o_ps)
                    nc.sync.dma_start(outv[:, mo8, ts(qt, NQ)], o_sb[:])
    return nc


def _get_program(mask):
    cls, qoff, mixed_idx, mixed_tiles, use_affine = _classify_mask(mask)
    key = (use_affine,
           tuple(tuple(r) for r in cls),
           tuple(tuple(r) for r in qoff))
    if key not in _cache:
        nc = _build_program(cls, qoff, mixed_idx, len(mixed_tiles), use_affine)
        nc.compile()
        _cache[key] = nc
    return _cache[key], mixed_tiles


def _prep_in_maps(x, mask, Wq, bq, Wk, bk, Wv, bv, Wo, bo, mixed_tiles):
    xT = [np.ascontiguousarray(x[b].T).astype(BF16) for b in range(B)]
    in_maps = []
    for core in range(NCORES):
        b, g = divmod(core, GROUPS)
        c0, c1 = g * DG, (g + 1) * DG
        im = {
            "xT": xT[b],
            "wq": np.ascontiguousarray(Wq[:, c0:c1] * SCALE).astype(BF16),
            "wk": np.ascontiguousarray(Wk[:, c0:c1]).astype(BF16),
            "wv": np.ascontiguousarray(Wv[:, c0:c1]).astype(BF16),
            "wo": np.ascontiguousarray(Wo[c0:c1, :]).astype(BF16),
            "bqk": np.ascontiguousarray(
                np.stack([bq[c0:c1] * SCALE, bk[c0:c1]])).astype(np.float32),
        }
        if len(mixed_tiles):
            im["mmask"] = mixed_tiles
        in_maps.append(im)
    return in_maps


def _unshard(results, Wo, bv, bo):
    bo_eff = (bo.astype(np.float32)
              + bv.astype(np.float32) @ Wo.astype(np.float32))
    out = np.empty((B, S, D), np.float32)
    for b in range(B):
        acc = results[b * GROUPS]["outT"].astype(np.float32).copy()
        for g in range(1, GROUPS):
            acc += results[b * GROUPS + g]["outT"]
        out[b] = acc.T + bo_eff
    return out


def kernel(trace=False, **inputs):
    from concourse import bass_utils

    args = {k: np.asarray(v) for k, v in inputs.items()}
    x, mask = args["x"], args["mask"]
    Wq, bq = args["Wq"], args["bq"]
    Wk, bk = args["Wk"], args["bk"]
    Wv, bv = args["Wv"], args["bv"]
    Wo, bo = args["Wo"], args["bo"]

    nc, mixed_tiles = _get_program(mask)
    in_maps = _prep_in_maps(x, mask, Wq, bq, Wk, bk, Wv, bv, Wo, bo,
                            mixed_tiles)
    res = bass_utils.run_bass_kernel_spmd(
        nc, in_maps, core_ids=list(range(NCORES)), trace=trace)
    out = _unshard(res.results, Wo, bv, bo)
    kernel.last_results = res
    return out
